# revision 31
# baseline (speedup 1.0000x reference)
"""MHA kernel for Trainium2, 8 NeuronCores.

Sharding: 4-way data parallel over batch x 2-way Megatron tensor parallel
over heads (8 heads / 512 dims per core). Wq/Wk/Wv split column-wise,
Wo split row-wise; the two TP partial outputs per batch are summed on host.

Per-core layout: everything runs "head-transposed" (Q^T, K^T as [d, s]),
so attention scores are computed directly as S^T = K Q^T with keys on
partitions.  A ones-column appended to V gives the softmax denominators
as a free 65th output row of the attention-value matmul.  RoPE is applied
in the transposed layout with a pair-rotation matmul on the PE plus
elementwise multiplies against bf16 cos/sin tables.

Pipeline structure (v2): K-projection+rope runs up front; the Q
projection is produced per 512-query tile and interleaved with the
attention loop, and the output projection consumes each query tile as
soon as all 4 head-pairs finish it.  Attention per (pair, qtile) walks
key chunks with double-buffered PSUM score slots so the PE streams
matmuls continuously while ScalarE exponentiates the previous chunk.
Softmax normalization uses the custom-DVE fast reciprocal plus a GpSimd
partition broadcast (no PE broadcast matmuls, no big DVE reciprocals).
"""

import math
import sys

sys.path.insert(0, "/opt/trn_rl_repo")

import numpy as np

B = 4
S = 2048
DM = 1024
NH = 16
TP = 2                # tensor-parallel ways
HD = DM // TP         # 512 head-dims per core
NHL = NH // TP        # 8 local heads
DK = 64
NPAIR = NHL // 2      # 4 local head pairs (one 128-row chunk each)
P = 128
KCH = S // P          # 16 key chunks
NQT = S // 512        # 4 query tiles
ROPE_THETA = 10000.0
SM_SCALE = 1.0 / math.sqrt(DK)

_CACHE = {}


def _build_nc():
    import concourse.bass as bass
    from concourse import mybir
    from concourse.tile import TileContext
    from contextlib import ExitStack

    f32 = mybir.dt.float32
    bf16 = mybir.dt.bfloat16
    EXP = mybir.ActivationFunctionType.Exp
    LN = mybir.ActivationFunctionType.Ln
    MUL = mybir.AluOpType.mult
    ADD = mybir.AluOpType.add

    nc = bass.Bass()
    xq = nc.declare_dram_parameter("xq_t", [DM, S], bf16, isOutput=False)
    xk = nc.declare_dram_parameter("xk_t", [DM, S], bf16, isOutput=False)
    xv = nc.declare_dram_parameter("xv_t", [DM, S], bf16, isOutput=False)
    wq = nc.declare_dram_parameter("wq", [DM, HD], bf16, isOutput=False)
    wk = nc.declare_dram_parameter("wk", [DM, HD], bf16, isOutput=False)
    wv = nc.declare_dram_parameter("wv", [DM, HD], bf16, isOutput=False)
    wo = nc.declare_dram_parameter("wo", [HD, DM], bf16, isOutput=False)
    cosd = nc.declare_dram_parameter("cos_t", [P, S], bf16, isOutput=False)
    sind = nc.declare_dram_parameter("sin_t", [P, S], bf16, isOutput=False)
    rotd = nc.declare_dram_parameter("rot_t", [P, P], bf16, isOutput=False)
    seld = nc.declare_dram_parameter("sel2", [65, P], bf16, isOutput=False)
    y = nc.declare_dram_parameter("y", [S, DM], f32, isOutput=True)

    with TileContext(nc) as tc, ExitStack() as top:
        persist = top.enter_context(tc.tile_pool(name="persist", bufs=1))
        vaug = persist.tile([P, KCH, NHL * 65], bf16)   # V + ones col per head
        q_rot = persist.tile([P, NPAIR, S], bf16)
        k_rot = persist.tile([P, NPAIR, S], bf16)
        o_norm = persist.tile([P, NPAIR, S], bf16)

        consts = top.enter_context(tc.tile_pool(name="consts", bufs=1))
        cos_sb = consts.tile([P, S], bf16)
        nc.sync.dma_start(out=cos_sb, in_=cosd[:, :])
        sin_sb = consts.tile([P, S], bf16)
        nc.sync.dma_start(out=sin_sb, in_=sind[:, :])
        rot_sb = consts.tile([P, P], bf16)
        nc.sync.dma_start(out=rot_sb, in_=rotd[:, :])
        wq_sb = consts.tile([P, DM // P, HD], bf16)
        nc.sync.dma_start(out=wq_sb, in_=wq.rearrange("(c p) n -> p c n", p=P))
        wk_sb = consts.tile([P, DM // P, HD], bf16)
        nc.sync.dma_start(out=wk_sb, in_=wk.rearrange("(c p) n -> p c n", p=P))
        wo_sb = consts.tile([P, NPAIR, DM], bf16)
        nc.sync.dma_start(out=wo_sb, in_=wo.rearrange("(c p) n -> p c n", p=P))
        sel2 = consts.tile([65, P], bf16)
        nc.sync.dma_start(out=sel2, in_=seld[:, :])
        # denominator scratch: heads at partitions 0 and 64; the unused
        # rows are memset once to 1.0 so ln/exp of them stay finite and
        # the selector matmul's zero weights see no NaNs.
        den_t = [consts.tile([65, 512], f32, name=f"den_t{i}")
                 for i in range(2)]
        ld_t = [consts.tile([65, 512], f32, name=f"ld_t{i}")
                for i in range(2)]
        rc_t = [consts.tile([65, 512], bf16, name=f"rc_t{i}")
                for i in range(2)]
        for t in den_t:
            nc.vector.memset(t, 1.0)

        for h in range(NHL):
            nc.vector.memset(vaug[:, :, 65 * h + 64: 65 * h + 65], 1.0)

        # ---------------- Phase A: V projection ----------------
        with tc.tile_pool(name="phA", bufs=1) as pA, \
                tc.tile_pool(name="phA_x", bufs=8) as pAx, \
                tc.tile_pool(name="phA_ps", bufs=3, space="PSUM") as pAps:
            wv_sb = pA.tile([P, DM // P, HD], bf16)
            nc.sync.dma_start(out=wv_sb, in_=wv.rearrange("(c p) n -> p c n", p=P))
            xv_sb = []
            for c in range(DM // P):
                t = pAx.tile([P, S], bf16)
                nc.sync.dma_start(out=t, in_=xv[c * P:(c + 1) * P, :])
                xv_sb.append(t)
            for sc in range(KCH):
                vps = pAps.tile([P, HD], f32)
                for c in range(DM // P):
                    nc.tensor.matmul(
                        vps,
                        lhsT=xv_sb[c][:, sc * P:(sc + 1) * P],
                        rhs=wv_sb[:, c, :],
                        start=(c == 0),
                        stop=(c == DM // P - 1),
                    )
                nc.vector.tensor_copy(
                    out=vaug[:, sc, :].rearrange("p (h e) -> p h e", e=65)[:, :, 0:64],
                    in_=vps.rearrange("p (h e) -> p h e", e=64),
                )

        # Q/K projection + RoPE for one 512-col seq tile, one head pair.
        # qps/rps share a single PSUM bank slot (tag via name=).
        # xt_at(c) returns the [P, 512] input chunk for dmodel chunk c.
        def proj_rope_m(pps, pt, xt_at, w_sb, dest, nt, m):
            ns = slice(nt * 512, (nt + 1) * 512)
            qps = pps.tile([P, 512], f32, name="bq")
            for c in range(DM // P):
                nc.tensor.matmul(
                    qps,
                    lhsT=w_sb[:, c, m * P:(m + 1) * P],
                    rhs=xt_at(c),
                    start=(c == 0),
                    stop=(c == DM // P - 1),
                )
            qp = pt.tile([P, 512], bf16, name="qp")
            nc.vector.tensor_copy(out=qp, in_=qps)
            rps = pps.tile([P, 512], f32, name="bq")
            nc.tensor.matmul(rps, lhsT=rot_sb, rhs=qp)
            t1 = pt.tile([P, 512], bf16, name="t1")
            nc.vector.tensor_tensor(t1, qp, cos_sb[:, ns], MUL)
            t2 = pt.tile([P, 512], bf16, name="t2")
            nc.vector.tensor_tensor(t2, rps, sin_sb[:, ns], MUL)
            nc.vector.tensor_tensor(dest[:, m, ns], t1, t2, ADD)

        # ---------------- Phase B-K: K projection + RoPE (all tiles) ----
        # m-major so pair 0's k_rot completes first and attention can
        # start while later pairs are still projecting.
        with tc.tile_pool(name="phBK_x", bufs=8) as pkx, \
                tc.tile_pool(name="phBK_ps", bufs=2, space="PSUM") as pkps, \
                tc.tile_pool(name="phBK_t", bufs=3) as pkt:
            xk_sb = []
            for c in range(DM // P):
                t = pkx.tile([P, S], bf16)
                nc.sync.dma_start(out=t, in_=xk[c * P:(c + 1) * P, :])
                xk_sb.append(t)
            for m in range(NPAIR):
                for nt in range(NQT):
                    ns = slice(nt * 512, (nt + 1) * 512)
                    proj_rope_m(pkps, pkt, lambda c, ns=ns: xk_sb[c][:, ns],
                                wk_sb, k_rot, nt, m)

        # ---------------- main loop: Q proj + attention + out proj ------
        with tc.tile_pool(name="phC_s", bufs=2, space="PSUM") as ps_s, \
                tc.tile_pool(name="phC_o", bufs=1, space="PSUM") as ps_o, \
                tc.tile_pool(name="phBQ_ps", bufs=1, space="PSUM") as pqps, \
                tc.tile_pool(name="aux_ps", bufs=1, space="PSUM") as paux, \
                tc.tile_pool(name="phBQ_x", bufs=16) as pqx, \
                tc.tile_pool(name="phBQ_t", bufs=3) as pqt, \
                tc.tile_pool(name="phC_e", bufs=3) as pe, \
                tc.tile_pool(name="phC_n", bufs=4) as pn, \
                tc.tile_pool(name="phD_y", bufs=2) as py:

            def load_xq(nt):
                ns = slice(nt * 512, (nt + 1) * 512)
                chunks = []
                for c in range(DM // P):
                    t = pqx.tile([P, 512], bf16, name="xq")
                    nc.sync.dma_start(out=t, in_=xq[c * P:(c + 1) * P, ns])
                    chunks.append(t)
                return chunks

            def bq_m(nt, chunks, m):
                proj_rope_m(pqps, pqt, lambda c: chunks[c], wq_sb, q_rot,
                            nt, m)

            def attn_dks(p, qt, oA, oB, dks):
                qs = slice(qt * 512, (qt + 1) * 512)
                hA = 2 * p
                hB = 2 * p + 1
                for dk in dks:
                    sA = ps_s.tile([P, 1024], f32, name="s")
                    sB = ps_s.tile([P, 1024], f32, name="s")
                    for h2 in range(2):
                        kc = 2 * dk + h2
                        kcs = slice(kc * P, (kc + 1) * P)
                        hs = slice(h2 * 512, (h2 + 1) * 512)
                        nc.tensor.matmul(
                            sA[:, hs],
                            lhsT=k_rot[0:64, p, kcs],
                            rhs=q_rot[0:64, p, qs],
                        )
                        nc.tensor.matmul(
                            sB[:, hs],
                            lhsT=k_rot[64:128, p, kcs],
                            rhs=q_rot[64:128, p, qs],
                        )
                    eA = pe.tile([P, 1024], bf16, name="e")
                    nc.scalar.activation(eA, sA, EXP, scale=SM_SCALE)
                    for h2 in range(2):
                        kc = 2 * dk + h2
                        hs = slice(h2 * 512, (h2 + 1) * 512)
                        nc.tensor.matmul(
                            oA,
                            lhsT=vaug[:, kc, 65 * hA:65 * hA + 65],
                            rhs=eA[:, hs],
                            start=(dk == 0 and h2 == 0),
                            stop=(dk == KCH // 2 - 1 and h2 == 1),
                        )
                    eB = pe.tile([P, 1024], bf16, name="e")
                    nc.scalar.activation(eB, sB, EXP, scale=SM_SCALE)
                    for h2 in range(2):
                        kc = 2 * dk + h2
                        hs = slice(h2 * 512, (h2 + 1) * 512)
                        nc.tensor.matmul(
                            oB,
                            lhsT=vaug[:, kc, 65 * hB:65 * hB + 65],
                            rhs=eB[:, hs],
                            start=(dk == 0 and h2 == 0),
                            stop=(dk == KCH // 2 - 1 and h2 == 1),
                        )

            def attn_epilogue(p, qt, oA, oB):
                # softmax denominators: 1/den = exp(-ln(den)) on ScalarE
                # (ln+exp share one ACT table set with the attention exps),
                # broadcast across partitions with a selector matmul.
                qs = slice(qt * 512, (qt + 1) * 512)
                par = (4 * qt + p) % 2
                den2, ld2, rc2 = den_t[par], ld_t[par], rc_t[par]
                nc.vector.tensor_copy(out=den2[0:1, :], in_=oA[64:65, :])
                nc.vector.tensor_copy(out=den2[64:65, :], in_=oB[64:65, :])
                nc.scalar.activation(ld2, den2, LN)
                nc.scalar.activation(rc2, ld2, EXP, scale=-1.0)
                # sel2 routes 1/denA to partitions 0-63, 1/denB to 64-127
                bcp = paux.tile([P, 512], f32, name="aux")
                nc.tensor.matmul(bcp, lhsT=sel2, rhs=rc2)
                bcs = pn.tile([P, 512], f32, name="bcs")
                nc.vector.tensor_copy(out=bcs, in_=bcp)
                nc.vector.tensor_tensor(
                    o_norm[0:64, p, qs], oA[0:64, :], bcs[0:64, :], MUL,
                )
                nc.vector.tensor_tensor(
                    o_norm[64:128, p, qs], oB[0:64, :], bcs[64:128, :], MUL,
                )

            def proj_out_group(qc, nh):
                yps = paux.tile([P, 512], f32, name="aux")
                for p in range(NPAIR):
                    nc.tensor.matmul(
                        yps,
                        lhsT=o_norm[:, p, qc * P:(qc + 1) * P],
                        rhs=wo_sb[:, p, nh * 512:(nh + 1) * 512],
                        start=(p == 0),
                        stop=(p == NPAIR - 1),
                    )
                ys = py.tile([P, 512], f32, name="ys")
                nc.vector.tensor_copy(out=ys, in_=yps)
                nc.sync.dma_start(
                    out=y[qc * P:(qc + 1) * P, nh * 512:(nh + 1) * 512],
                    in_=ys,
                )

            # main braid: attention for (p, qt) with the next tile's Q
            # projection and the previous tile's output projection woven
            # between half-sized attention chunks, so the PE always has
            # filler work while ScalarE streams exponentials.
            chunks = load_xq(0)
            for m in range(NPAIR):
                bq_m(0, chunks, m)
            for qt in range(NQT):
                if qt + 1 < NQT:
                    chunks = load_xq(qt + 1)
                dgroups = [(qc, nh) for qc in range(4 * qt - 4, 4 * qt)
                           for nh in range(DM // 512)] if qt > 0 else []
                for p in range(NPAIR):
                    oA = ps_o.tile([65, 512], f32, name="oA")
                    oB = ps_o.tile([65, 512], f32, name="oB")
                    attn_dks(p, qt, oA, oB, range(0, 4))
                    if qt + 1 < NQT:
                        bq_m(qt + 1, chunks, p)
                    attn_dks(p, qt, oA, oB, range(4, 8))
                    for qc, nh in dgroups[2 * p:2 * p + 2]:
                        proj_out_group(qc, nh)
                    attn_epilogue(p, qt, oA, oB)
            for qc in range(4 * NQT - 4, 4 * NQT):
                for nh in range(DM // 512):
                    proj_out_group(qc, nh)

    return nc


_CTRL_OPS = {"Drain", "EventSemaphore", "Nop"}


def _patch_bir_waits(bs, ctrl_lim=1, other_lim=1):
    """Split instructions with more sync-waits than this walrus build's CTRL
    struct supports: excess waits move onto wait-only Drain instructions
    inserted immediately before (waiting earlier is always safe)."""
    import orjson

    j = orjson.loads(bs)
    for f in j["functions"]:
        for b in f["blocks"]:
            out = []
            for i in b["instructions"]:
                si = i.get("sync_info")
                w = si.get("on_wait", []) if si else []
                lim = ctrl_lim if i.get("opcode") in _CTRL_OPS else other_lim
                if len(w) > lim:
                    extra, keep = w[:-lim], w[-lim:]
                    for k, ww in enumerate(extra):
                        out.append({
                            "debug": i.get("debug", 0),
                            "engine": i["engine"],
                            "ins": [], "outs": [],
                            "is_reset_sema": False,
                            "name": i["name"] + f"-ws{k}",
                            "opcode": "Drain",
                            "sync_info": {"on_update": [], "on_wait": [ww]},
                        })
                    si["on_wait"] = keep
                out.append(i)
            b["instructions"] = out
    return orjson.dumps(j)


def _tables():
    if "tables" not in _CACHE:
        import ml_dtypes

        bf = ml_dtypes.bfloat16
        inv_freq = (ROPE_THETA ** (
            -np.arange(0, DK, 2, dtype=np.float32) / np.float32(DK)
        )).astype(np.float32)
        ang = np.arange(S, dtype=np.float32)[:, None] * inv_freq[None, :]  # [S, 32]
        cos_sj = np.cos(ang).astype(np.float32).T        # [32, S]
        sin_sj = np.sin(ang).astype(np.float32).T
        cos_t = np.tile(np.repeat(cos_sj, 2, axis=0), (2, 1))  # [128, S]
        sin_t = np.tile(np.repeat(sin_sj, 2, axis=0), (2, 1))
        prot = np.zeros((P, P), dtype=np.float32)
        for j in range(P // 2):
            prot[2 * j, 2 * j + 1] = -1.0
            prot[2 * j + 1, 2 * j] = 1.0
        rot_t = np.ascontiguousarray(prot.T)
        sel2 = np.zeros((65, P), dtype=np.float32)
        sel2[0, 0:64] = 1.0
        sel2[64, 64:128] = 1.0
        sel2 = sel2.astype(bf)
        _CACHE["tables"] = (
            np.ascontiguousarray(cos_t).astype(bf),
            np.ascontiguousarray(sin_t).astype(bf),
            rot_t.astype(bf),
            sel2,
        )
    return _CACHE["tables"]


def _get_nc():
    if "nc" not in _CACHE:
        nc = _build_nc()
        orig = nc.to_json_bytes
        nc.to_json_bytes = lambda: _patch_bir_waits(orig())
        _CACHE["nc"] = nc
    return _CACHE["nc"]


def _run(in_maps, **kwargs):
    from concourse.bass_utils import run_bass_kernel_spmd

    return run_bass_kernel_spmd(_get_nc(), in_maps, core_ids=list(range(B * TP)),
                                **kwargs)


def _make_in_maps(q, k, v, Wq, Wk, Wv, Wo):
    import ml_dtypes

    cos_t, sin_t, rot_t, sel2 = _tables()
    bf = ml_dtypes.bfloat16
    f = np.float32
    xt = {}
    for b in range(B):
        xt[b] = (
            np.ascontiguousarray(np.asarray(q[b], f).T).astype(bf),
            np.ascontiguousarray(np.asarray(k[b], f).T).astype(bf),
            np.ascontiguousarray(np.asarray(v[b], f).T).astype(bf),
        )
    wq_s, wk_s, wv_s, wo_s = {}, {}, {}, {}
    for t in range(TP):
        cs = slice(t * HD, (t + 1) * HD)
        wq_s[t] = np.ascontiguousarray(np.asarray(Wq, f)[:, cs]).astype(bf)
        wk_s[t] = np.ascontiguousarray(np.asarray(Wk, f)[:, cs]).astype(bf)
        wv_s[t] = np.ascontiguousarray(np.asarray(Wv, f)[:, cs]).astype(bf)
        wo_s[t] = np.ascontiguousarray(np.asarray(Wo, f)[cs, :]).astype(bf)
    in_maps = []
    for core in range(B * TP):
        b, t = divmod(core, TP)
        in_maps.append({
            "xq_t": xt[b][0],
            "xk_t": xt[b][1],
            "xv_t": xt[b][2],
            "wq": wq_s[t],
            "wk": wk_s[t],
            "wv": wv_s[t],
            "wo": wo_s[t],
            "cos_t": cos_t,
            "sin_t": sin_t,
            "rot_t": rot_t,
            "sel2": sel2,
        })
    return in_maps


def kernel(q, k, v, Wq, Wk, Wv, Wo):
    res = _run(_make_in_maps(q, k, v, Wq, Wk, Wv, Wo))
    out = np.zeros((B, S, DM), np.float32)
    for core in range(B * TP):
        out[core // TP] += res.results[core]["y"]
    return out


# revision 34
# speedup vs baseline: 1.2002x; 1.2002x over previous
"""MHA kernel for Trainium2, 8 NeuronCores.

Sharding: 4-way data parallel over batch x 2-way Megatron tensor parallel
over heads (8 heads / 512 dims per core). Wq/Wk/Wv split column-wise,
Wo split row-wise; the two TP partial outputs per batch are summed on host.

Per-core layout: everything runs "head-transposed" (Q^T, K^T as [d, s]),
so attention scores are computed directly as S^T = K Q^T with keys on
partitions.  A ones-column appended to V gives the softmax denominators
as a free 65th output row of the attention-value matmul.  RoPE is applied
in the transposed layout with a pair-rotation matmul on the PE plus
elementwise multiplies against bf16 cos/sin tables.

Pipeline structure (v2): K-projection+rope runs up front; the Q
projection is produced per 512-query tile and interleaved with the
attention loop, and the output projection consumes each query tile as
soon as all 4 head-pairs finish it.  Attention per (pair, qtile) walks
key chunks with double-buffered PSUM score slots so the PE streams
matmuls continuously while ScalarE exponentiates the previous chunk.
Softmax normalization uses the custom-DVE fast reciprocal plus a GpSimd
partition broadcast (no PE broadcast matmuls, no big DVE reciprocals).
"""

import math
import sys

sys.path.insert(0, "/opt/trn_rl_repo")

import numpy as np

B = 4
S = 2048
DM = 1024
NH = 16
TP = 2                # tensor-parallel ways
HD = DM // TP         # 512 head-dims per core
NHL = NH // TP        # 8 local heads
DK = 64
NPAIR = NHL // 2      # 4 local head pairs (one 128-row chunk each)
P = 128
KCH = S // P          # 16 key chunks
NQT = S // 512        # 4 query tiles
ROPE_THETA = 10000.0
SM_SCALE = 1.0 / math.sqrt(DK)

_CACHE = {}


def _build_nc():
    import concourse.bass as bass
    from concourse import mybir
    from concourse.tile import TileContext
    from contextlib import ExitStack

    f32 = mybir.dt.float32
    bf16 = mybir.dt.bfloat16
    EXP = mybir.ActivationFunctionType.Exp
    LN = mybir.ActivationFunctionType.Ln
    MUL = mybir.AluOpType.mult
    ADD = mybir.AluOpType.add

    nc = bass.Bass()
    xq = nc.declare_dram_parameter("xq_t", [DM, S], bf16, isOutput=False)
    xk = nc.declare_dram_parameter("xk_t", [DM, S], bf16, isOutput=False)
    xv = nc.declare_dram_parameter("xv_t", [DM, S], bf16, isOutput=False)
    wq = nc.declare_dram_parameter("wq", [DM, HD], bf16, isOutput=False)
    wk = nc.declare_dram_parameter("wk", [DM, HD], bf16, isOutput=False)
    wv = nc.declare_dram_parameter("wv", [DM, HD], bf16, isOutput=False)
    wo = nc.declare_dram_parameter("wo", [HD, DM], bf16, isOutput=False)
    cosd = nc.declare_dram_parameter("cos_t", [P, S], bf16, isOutput=False)
    sind = nc.declare_dram_parameter("sin_t", [P, S], bf16, isOutput=False)
    rotd = nc.declare_dram_parameter("rot_t", [P, P], bf16, isOutput=False)
    seld = nc.declare_dram_parameter("sel2", [65, P], bf16, isOutput=False)
    y = nc.declare_dram_parameter("y", [S, DM], f32, isOutput=True)

    with TileContext(nc) as tc, ExitStack() as top:
        persist = top.enter_context(tc.tile_pool(name="persist", bufs=1))
        vaug = persist.tile([P, KCH, NHL * 65], bf16)   # V + ones col per head
        q_rot = persist.tile([P, NPAIR, S], bf16)
        k_rot = persist.tile([P, NPAIR, S], bf16)
        o_norm = persist.tile([P, NPAIR, S], bf16)

        consts = top.enter_context(tc.tile_pool(name="consts", bufs=1))
        cos_sb = consts.tile([P, S], bf16)
        nc.sync.dma_start(out=cos_sb, in_=cosd[:, :])
        sin_sb = consts.tile([P, S], bf16)
        nc.sync.dma_start(out=sin_sb, in_=sind[:, :])
        rot_sb = consts.tile([P, P], bf16)
        nc.sync.dma_start(out=rot_sb, in_=rotd[:, :])
        wq_sb = consts.tile([P, DM // P, HD], bf16)
        nc.sync.dma_start(out=wq_sb, in_=wq.rearrange("(c p) n -> p c n", p=P))
        wk_sb = consts.tile([P, DM // P, HD], bf16)
        nc.sync.dma_start(out=wk_sb, in_=wk.rearrange("(c p) n -> p c n", p=P))
        wo_sb = consts.tile([P, NPAIR, DM], bf16)
        nc.sync.dma_start(out=wo_sb, in_=wo.rearrange("(c p) n -> p c n", p=P))
        sel2 = consts.tile([65, P], bf16)
        nc.sync.dma_start(out=sel2, in_=seld[:, :])
        # denominator scratch: heads at partitions 0 and 64; the unused
        # rows are memset once to 1.0 so ln/exp of them stay finite and
        # the selector matmul's zero weights see no NaNs.
        den_t = [consts.tile([65, 512], f32, name=f"den_t{i}")
                 for i in range(2)]
        ld_t = [consts.tile([65, 512], f32, name=f"ld_t{i}")
                for i in range(2)]
        rc_t = [consts.tile([65, 512], bf16, name=f"rc_t{i}")
                for i in range(2)]
        for t in den_t:
            nc.vector.memset(t, 1.0)

        for h in range(NHL):
            nc.vector.memset(vaug[:, :, 65 * h + 64: 65 * h + 65], 1.0)

        # ---------------- Phase A: V projection ----------------
        # A's PSUM pool closes after phase A; the o-accumulator pool is
        # opened LAST in the main block so it lands on A's released banks
        # (the first attention-value matmul needs vaug anyway, so the
        # bank release adds no stall).
        with tc.tile_pool(name="phA", bufs=1) as pA, \
                tc.tile_pool(name="phA_x", bufs=8) as pAx, \
                tc.tile_pool(name="phA_ps", bufs=2, space="PSUM") as pAps:
            wv_sb = pA.tile([P, DM // P, HD], bf16)
            nc.sync.dma_start(out=wv_sb, in_=wv.rearrange("(c p) n -> p c n", p=P))
            xv_sb = []
            for c in range(DM // P):
                t = pAx.tile([P, S], bf16)
                nc.sync.dma_start(out=t, in_=xv[c * P:(c + 1) * P, :])
                xv_sb.append(t)
            for sc in range(KCH):
                vps = pAps.tile([P, HD], f32)
                for c in range(DM // P):
                    nc.tensor.matmul(
                        vps,
                        lhsT=xv_sb[c][:, sc * P:(sc + 1) * P],
                        rhs=wv_sb[:, c, :],
                        start=(c == 0),
                        stop=(c == DM // P - 1),
                    )
                nc.vector.tensor_copy(
                    out=vaug[:, sc, :].rearrange("p (h e) -> p h e", e=65)[:, :, 0:64],
                    in_=vps.rearrange("p (h e) -> p h e", e=64),
                )

        # ---------------- main block: K/Q proj + attention + out proj ---
        with tc.tile_pool(name="proj_ps", bufs=1, space="PSUM") as pps, \
                tc.tile_pool(name="phC_s", bufs=2, space="PSUM") as ps_s, \
                tc.tile_pool(name="aux_ps", bufs=1, space="PSUM") as paux, \
                tc.tile_pool(name="phC_o", bufs=1, space="PSUM") as ps_o, \
                tc.tile_pool(name="phBK_x", bufs=8) as pkx, \
                tc.tile_pool(name="phBQ_x", bufs=16) as pqx, \
                tc.tile_pool(name="proj_t", bufs=2) as pt, \
                tc.tile_pool(name="phC_e", bufs=3) as pe, \
                tc.tile_pool(name="phC_n", bufs=2) as pn, \
                tc.tile_pool(name="phD_y", bufs=2) as py:

            xk_sb = []
            for c in range(DM // P):
                t = pkx.tile([P, S], bf16)
                nc.sync.dma_start(out=t, in_=xk[c * P:(c + 1) * P, :])
                xk_sb.append(t)

            # Q/K projection + RoPE for one 512-col seq tile, one pair.
            # qps/rps share a single PSUM bank slot (tag via name=).
            def proj_rope_m(xt_at, w_sb, dest, nt, m):
                ns = slice(nt * 512, (nt + 1) * 512)
                qps = pps.tile([P, 512], f32, name="bq")
                for c in range(DM // P):
                    nc.tensor.matmul(
                        qps,
                        lhsT=w_sb[:, c, m * P:(m + 1) * P],
                        rhs=xt_at(c),
                        start=(c == 0),
                        stop=(c == DM // P - 1),
                    )
                qp = pt.tile([P, 512], bf16, name="qp")
                nc.vector.tensor_copy(out=qp, in_=qps)
                rps = pps.tile([P, 512], f32, name="bq")
                nc.tensor.matmul(rps, lhsT=rot_sb, rhs=qp)
                t1 = pt.tile([P, 512], bf16, name="t1")
                nc.vector.tensor_tensor(t1, qp, cos_sb[:, ns], MUL)
                t2 = pt.tile([P, 512], bf16, name="t2")
                nc.vector.tensor_tensor(t2, rps, sin_sb[:, ns], MUL)
                nc.vector.tensor_tensor(dest[:, m, ns], t1, t2, ADD)

            def bk_m(m):
                for nt in range(NQT):
                    ns = slice(nt * 512, (nt + 1) * 512)
                    proj_rope_m(lambda c, ns=ns: xk_sb[c][:, ns],
                                wk_sb, k_rot, nt, m)

            def load_xq(nt):
                ns = slice(nt * 512, (nt + 1) * 512)
                chunks = []
                for c in range(DM // P):
                    t = pqx.tile([P, 512], bf16, name="xq")
                    nc.sync.dma_start(out=t, in_=xq[c * P:(c + 1) * P, ns])
                    chunks.append(t)
                return chunks

            def bq_m(nt, chunks, m):
                proj_rope_m(lambda c: chunks[c], wq_sb, q_rot, nt, m)

            def attn_dks(p, qt, oA, oB, dks):
                qs = slice(qt * 512, (qt + 1) * 512)
                hA = 2 * p
                hB = 2 * p + 1
                for dk in dks:
                    sA = ps_s.tile([P, 1024], f32, name="s")
                    sB = ps_s.tile([P, 1024], f32, name="s")
                    for h2 in range(2):
                        kc = 2 * dk + h2
                        kcs = slice(kc * P, (kc + 1) * P)
                        hs = slice(h2 * 512, (h2 + 1) * 512)
                        nc.tensor.matmul(
                            sA[:, hs],
                            lhsT=k_rot[0:64, p, kcs],
                            rhs=q_rot[0:64, p, qs],
                        )
                        nc.tensor.matmul(
                            sB[:, hs],
                            lhsT=k_rot[64:128, p, kcs],
                            rhs=q_rot[64:128, p, qs],
                        )
                    eA = pe.tile([P, 1024], bf16, name="e")
                    nc.scalar.activation(eA, sA, EXP, scale=SM_SCALE)
                    for h2 in range(2):
                        kc = 2 * dk + h2
                        hs = slice(h2 * 512, (h2 + 1) * 512)
                        nc.tensor.matmul(
                            oA,
                            lhsT=vaug[:, kc, 65 * hA:65 * hA + 65],
                            rhs=eA[:, hs],
                            start=(dk == 0 and h2 == 0),
                            stop=(dk == KCH // 2 - 1 and h2 == 1),
                        )
                    eB = pe.tile([P, 1024], bf16, name="e")
                    nc.scalar.activation(eB, sB, EXP, scale=SM_SCALE)
                    for h2 in range(2):
                        kc = 2 * dk + h2
                        hs = slice(h2 * 512, (h2 + 1) * 512)
                        nc.tensor.matmul(
                            oB,
                            lhsT=vaug[:, kc, 65 * hB:65 * hB + 65],
                            rhs=eB[:, hs],
                            start=(dk == 0 and h2 == 0),
                            stop=(dk == KCH // 2 - 1 and h2 == 1),
                        )

            def attn_epilogue(p, qt, oA, oB):
                # softmax denominators: 1/den = exp(-ln(den)) on ScalarE
                # (ln+exp share one ACT table set with the attention exps),
                # broadcast across partitions with a selector matmul.
                qs = slice(qt * 512, (qt + 1) * 512)
                par = (4 * qt + p) % 2
                den2, ld2, rc2 = den_t[par], ld_t[par], rc_t[par]
                nc.vector.tensor_copy(out=den2[0:1, :], in_=oA[64:65, :])
                nc.vector.tensor_copy(out=den2[64:65, :], in_=oB[64:65, :])
                nc.scalar.activation(ld2, den2, LN)
                nc.scalar.activation(rc2, ld2, EXP, scale=-1.0)
                # sel2 routes 1/denA to partitions 0-63, 1/denB to 64-127
                bcp = paux.tile([P, 512], f32, name="aux")
                nc.tensor.matmul(bcp, lhsT=sel2, rhs=rc2)
                bcs = pn.tile([P, 512], f32, name="bcs")
                nc.vector.tensor_copy(out=bcs, in_=bcp)
                nc.vector.tensor_tensor(
                    o_norm[0:64, p, qs], oA[0:64, :], bcs[0:64, :], MUL,
                )
                nc.vector.tensor_tensor(
                    o_norm[64:128, p, qs], oB[0:64, :], bcs[64:128, :], MUL,
                )

            def proj_out_group(qc, nh):
                yps = paux.tile([P, 512], f32, name="aux")
                for p in range(NPAIR):
                    nc.tensor.matmul(
                        yps,
                        lhsT=o_norm[:, p, qc * P:(qc + 1) * P],
                        rhs=wo_sb[:, p, nh * 512:(nh + 1) * 512],
                        start=(p == 0),
                        stop=(p == NPAIR - 1),
                    )
                ys = py.tile([P, 512], f32, name="ys")
                nc.vector.tensor_copy(out=ys, in_=yps)
                nc.sync.dma_start(
                    out=y[qc * P:(qc + 1) * P, nh * 512:(nh + 1) * 512],
                    in_=ys,
                )

            # main braid: attention for (p, qt) with K/Q projection and
            # output-projection groups woven between half-sized attention
            # chunks, so the PE always has filler work while ScalarE
            # streams exponentials.  qt=0 weaves the remaining K pairs'
            # projections (pair 0's K/Q runs up front).
            chunks = load_xq(0)
            bk_m(0)
            bq_m(0, chunks, 0)
            for qt in range(NQT):
                if qt + 1 < NQT:
                    nxt = load_xq(qt + 1)
                dgroups = [(qc, nh) for qc in range(4 * qt - 4, 4 * qt)
                           for nh in range(DM // 512)] if qt > 0 else []
                for p in range(NPAIR):
                    oA = ps_o.tile([65, 512], f32, name="oA")
                    oB = ps_o.tile([65, 512], f32, name="oB")
                    attn_dks(p, qt, oA, oB, range(0, 4))
                    if qt == 0 and p + 1 < NPAIR:
                        bk_m(p + 1)
                    attn_dks(p, qt, oA, oB, range(4, 8))
                    attn_epilogue(p, qt, oA, oB)
                    if qt == 0 and p + 1 < NPAIR:
                        bq_m(0, chunks, p + 1)
                    if qt + 1 < NQT:
                        bq_m(qt + 1, nxt, p)
                    for qc, nh in dgroups[2 * p:2 * p + 2]:
                        proj_out_group(qc, nh)
            for qc in range(4 * NQT - 4, 4 * NQT):
                for nh in range(DM // 512):
                    proj_out_group(qc, nh)

    return nc


_CTRL_OPS = {"Drain", "EventSemaphore", "Nop"}


def _patch_bir_waits(bs, ctrl_lim=1, other_lim=1):
    """Split instructions with more sync-waits than this walrus build's CTRL
    struct supports: excess waits move onto wait-only Drain instructions
    inserted immediately before (waiting earlier is always safe)."""
    import orjson

    j = orjson.loads(bs)
    for f in j["functions"]:
        for b in f["blocks"]:
            out = []
            for i in b["instructions"]:
                si = i.get("sync_info")
                w = si.get("on_wait", []) if si else []
                lim = ctrl_lim if i.get("opcode") in _CTRL_OPS else other_lim
                if len(w) > lim:
                    extra, keep = w[:-lim], w[-lim:]
                    for k, ww in enumerate(extra):
                        out.append({
                            "debug": i.get("debug", 0),
                            "engine": i["engine"],
                            "ins": [], "outs": [],
                            "is_reset_sema": False,
                            "name": i["name"] + f"-ws{k}",
                            "opcode": "Drain",
                            "sync_info": {"on_update": [], "on_wait": [ww]},
                        })
                    si["on_wait"] = keep
                out.append(i)
            b["instructions"] = out
    return orjson.dumps(j)


def _tables():
    if "tables" not in _CACHE:
        import ml_dtypes

        bf = ml_dtypes.bfloat16
        inv_freq = (ROPE_THETA ** (
            -np.arange(0, DK, 2, dtype=np.float32) / np.float32(DK)
        )).astype(np.float32)
        ang = np.arange(S, dtype=np.float32)[:, None] * inv_freq[None, :]  # [S, 32]
        cos_sj = np.cos(ang).astype(np.float32).T        # [32, S]
        sin_sj = np.sin(ang).astype(np.float32).T
        cos_t = np.tile(np.repeat(cos_sj, 2, axis=0), (2, 1))  # [128, S]
        sin_t = np.tile(np.repeat(sin_sj, 2, axis=0), (2, 1))
        prot = np.zeros((P, P), dtype=np.float32)
        for j in range(P // 2):
            prot[2 * j, 2 * j + 1] = -1.0
            prot[2 * j + 1, 2 * j] = 1.0
        rot_t = np.ascontiguousarray(prot.T)
        sel2 = np.zeros((65, P), dtype=np.float32)
        sel2[0, 0:64] = 1.0
        sel2[64, 64:128] = 1.0
        sel2 = sel2.astype(bf)
        _CACHE["tables"] = (
            np.ascontiguousarray(cos_t).astype(bf),
            np.ascontiguousarray(sin_t).astype(bf),
            rot_t.astype(bf),
            sel2,
        )
    return _CACHE["tables"]


def _get_nc():
    if "nc" not in _CACHE:
        nc = _build_nc()
        orig = nc.to_json_bytes
        nc.to_json_bytes = lambda: _patch_bir_waits(orig())
        _CACHE["nc"] = nc
    return _CACHE["nc"]


def _run(in_maps, **kwargs):
    from concourse.bass_utils import run_bass_kernel_spmd

    return run_bass_kernel_spmd(_get_nc(), in_maps, core_ids=list(range(B * TP)),
                                **kwargs)


def _make_in_maps(q, k, v, Wq, Wk, Wv, Wo):
    import ml_dtypes

    cos_t, sin_t, rot_t, sel2 = _tables()
    bf = ml_dtypes.bfloat16
    f = np.float32
    xt = {}
    for b in range(B):
        xt[b] = (
            np.ascontiguousarray(np.asarray(q[b], f).T).astype(bf),
            np.ascontiguousarray(np.asarray(k[b], f).T).astype(bf),
            np.ascontiguousarray(np.asarray(v[b], f).T).astype(bf),
        )
    wq_s, wk_s, wv_s, wo_s = {}, {}, {}, {}
    for t in range(TP):
        cs = slice(t * HD, (t + 1) * HD)
        wq_s[t] = np.ascontiguousarray(np.asarray(Wq, f)[:, cs]).astype(bf)
        wk_s[t] = np.ascontiguousarray(np.asarray(Wk, f)[:, cs]).astype(bf)
        wv_s[t] = np.ascontiguousarray(np.asarray(Wv, f)[:, cs]).astype(bf)
        wo_s[t] = np.ascontiguousarray(np.asarray(Wo, f)[cs, :]).astype(bf)
    in_maps = []
    for core in range(B * TP):
        b, t = divmod(core, TP)
        in_maps.append({
            "xq_t": xt[b][0],
            "xk_t": xt[b][1],
            "xv_t": xt[b][2],
            "wq": wq_s[t],
            "wk": wk_s[t],
            "wv": wv_s[t],
            "wo": wo_s[t],
            "cos_t": cos_t,
            "sin_t": sin_t,
            "rot_t": rot_t,
            "sel2": sel2,
        })
    return in_maps


def kernel(q, k, v, Wq, Wk, Wv, Wo):
    res = _run(_make_in_maps(q, k, v, Wq, Wk, Wv, Wo))
    out = np.zeros((B, S, DM), np.float32)
    for core in range(B * TP):
        out[core // TP] += res.results[core]["y"]
    return out
